# revision 12
# baseline (speedup 1.0000x reference)
"""Trainium2 Bass kernel for the attention block (QKV proj + RMSNorm +
RoPE + softmax attention + out proj), batch-parallel across 8 cores.

Design notes:
  - bf16 staging for x/w/q/k/v/P/outT (DVE 2x perf modes, 1.0 cyc/row
    transposes, halved DMA); psum accumulation stays f32. Host pre-swizzles
    every input into one-contiguous-descriptor-per-partition layouts and
    folds the rms-norm weights into the RoPE tables.
  - RMS rsqrt factors with ZERO activation-table switches: bit-hack log2
    on DVE (int32 bitcast - bias), one Exp on the attention's own act
    table, one Newton step; k's 1/sqrt(hd) softmax scale is folded into
    the Newton constants. The whole kernel loads one ACT table, once.
  - (128,1024) exp tiles (one per j-block) with per-partition fk scale;
    S/PV matmuls at N=512 (psum-bank limit).
  - a single shared (128,1024)-f32 psum ring (2 tiles = 4 banks) backs S
    tiles, wave-B qkv psums (2 lb-units per tile), wave-B transposes
    (bitcast bf16 view), and proj psums; ps_o (65,1024) x2 = 4 banks.
  - globally software-pipelined schedule: wave A (heads 0-7 qkv, ACT
    evacuations) -> transposes -> [attn pairs 0-3 interleaved with wave B
    qkv (DVE evacuations) + pair-4 transposes] -> [attn pairs 4-7 with
    pair 5-7 transposes as prefixes, interleaved with proj cb0-3
    partials into yacc] -> proj cb4-7 + combine. The S/exp stream runs
    2 j-blocks ahead of PV ACROSS head boundaries; kz staging is
    prefetched a head ahead; each head's normalization chain issues
    right after its last PV so the pso psum bank frees in time.
"""
import numpy as np
import ml_dtypes

import concourse.bass as bass
from concourse import bacc
import concourse.mybir as mybir
import concourse.tile as tile
from concourse.bass_utils import run_bass_kernel_spmd
from concourse.masks import make_identity

F32 = mybir.dt.float32
BF16 = mybir.dt.bfloat16
AF = mybir.ActivationFunctionType
ALU = mybir.AluOpType

B, L, C, H, HD = 8, 1024, 1024, 16, 64
EPS = 1e-6
NLB = L // 128
NCB = C // 128
NJB = L // 128
NPAIR = H // 2
N_CORES = 8
ACT_TABLE_LN_EXP = 6   # act_info.json act_func_sets index: natural_log_exp_and_others

_nc_cache = None
_last_results = None


def _bcast(ap2d, reps):
    """(128, w) AP -> (128, reps, w) stride-0 broadcast view."""
    return bass.AP(tensor=ap2d.tensor, offset=ap2d.offset,
                   ap=[ap2d.ap[0], [0, reps], ap2d.ap[1]])


def _sub(ap3d, lo, w):
    """(128, reps, 64) bcast view -> free-dim slice [lo:lo+w]."""
    return bass.AP(tensor=ap3d.tensor, offset=ap3d.offset + lo,
                   ap=[ap3d.ap[0], ap3d.ap[1], [1, w]])


def _inner_bcast(ap2d, w):
    """(128, n) AP -> (128, n, w) view broadcasting each element w times."""
    return bass.AP(tensor=ap2d.tensor, offset=ap2d.offset,
                   ap=[ap2d.ap[0], ap2d.ap[1], [0, w]])


DEBUG_DUMP = False


def build_nc():
    nc = bacc.Bacc("TRN2", target_bir_lowering=False)

    # host-swizzled layouts: one contiguous descriptor per partition row
    xT = nc.declare_dram_parameter("xT", [128, NCB, L], BF16, isOutput=False)
    wq = nc.declare_dram_parameter("wq", [128, 6, NCB, 512], BF16,
                                   isOutput=False)
    wp = nc.declare_dram_parameter("wp", [128, 2, NCB, 512], BF16,
                                   isOutput=False)
    cq = nc.declare_dram_parameter("cq", [128, NLB, HD], BF16, isOutput=False)
    sq = nc.declare_dram_parameter("sq", [128, NLB, HD], BF16, isOutput=False)
    ck = nc.declare_dram_parameter("ck", [128, NLB, HD], BF16, isOutput=False)
    sk = nc.declare_dram_parameter("sk", [128, NLB, HD], BF16, isOutput=False)
    y = nc.declare_dram_parameter("y", [L, C], F32, isOutput=True)
    dbg = {}
    if DEBUG_DUMP:
        for nm, shape, dt in [
                ("d_qr", [128, NLB, C], BF16), ("d_kr", [128, NLB, C], BF16),
                ("d_qT", [128, NPAIR, L], BF16), ("d_kT", [128, NPAIR, L], BF16),
                ("d_vb", [128, NJB, H, HD + 1], BF16),
                ("d_fk", [128, NLB, H], F32), ("d_outT", [128, NCB, L], BF16),
                ("d_yacc", [128, 2, NLB, 512], BF16),
                ("d_kz0", [128, L], BF16), ("d_kz1", [128, L], BF16),
                ("d_sT", [128, L], F32), ("d_pt", [128, L], BF16),
                ("d_pso", [HD + 1, L], F32), ("d_rs", [1, L], F32)]:
            dbg[nm] = nc.declare_dram_parameter(nm, shape, dt, isOutput=True)



    with tile.TileContext(nc) as tc:
        with tc.tile_pool(name="persist", bufs=1) as persist, \
             tc.tile_pool(name="stage", bufs=1) as stg:
            # ---------------- persistent tiles + prolog DMAs ----------------
            xr = persist.tile([128, NCB, L], BF16)
            nc.sync.dma_start(out=xr, in_=xT[:])

            cq_sb = persist.tile([128, NLB, HD], BF16)
            sq_sb = persist.tile([128, NLB, HD], BF16)
            ck_sb = persist.tile([128, NLB, HD], BF16)
            sk_sb = persist.tile([128, NLB, HD], BF16)

            def table_dmas():
                # issued after wqn0 so the first-matmul critical DMAs
                # (xr + wqn0) own the queues at startup
                nc.sync.dma_start(out=cq_sb, in_=cq[:])
                nc.sync.dma_start(out=sq_sb, in_=sq[:])
                nc.sync.dma_start(out=ck_sb, in_=ck[:])
                nc.sync.dma_start(out=sk_sb, in_=sk[:])

            fk_all = persist.tile([128, NLB, H], F32)
            vb = persist.tile([128, NJB, H, HD + 1], BF16)
            ident_f = persist.tile([128, 128], F32)
            make_identity(nc, ident_f)
            ident = persist.tile([128, 128], BF16)
            nc.vector.tensor_copy(ident, ident_f)
            ones128 = persist.tile([128, 1], F32)
            nc.vector.memset(ones128, 1.0)
            eps_q = persist.tile([128, 1], F32)
            nc.vector.memset(eps_q, EPS)
            eps_k = persist.tile([128, 1], F32)
            nc.vector.memset(eps_k, HD * EPS)
            zero_b = persist.tile([128, 1], F32)
            nc.vector.memset(zero_b, 0.0)
            nc.vector.tensor_copy(
                bass.AP(tensor=vb.tensor, offset=vb.offset + HD,
                        ap=[vb.ap[0], vb.ap[1], vb.ap[2], [1, 1]]),
                bass.AP(tensor=ones128.tensor, offset=ones128.offset,
                        ap=[ones128.ap[0], [0, NJB], [0, H], [1, 1]]))

            qr = persist.tile([128, NLB, C], BF16)
            kr = persist.tile([128, NLB, C], BF16)
            qT = persist.tile([128, NPAIR, L], BF16)
            kT = persist.tile([128, NPAIR, L], BF16)
            outT = persist.tile([128, NCB, L], BF16)
            yacc = persist.tile([128, 2, NLB, 512], BF16)  # proj cb0-3 partials

            # zero-padded kz tiles: head i data in rows [64i,64i+64), rest 0
            kz0 = persist.tile([128, L], BF16)
            kz1 = persist.tile([128, L], BF16)
            nc.vector.memset(kz0[HD:128, :], 0.0)
            nc.vector.memset(kz1[0:HD, :], 0.0)

            # ---------------- emission helpers ----------------
            def rope(src3, dst3, cos_sb, sin_sb, lb):
                cw = _bcast(cos_sb[:, lb, :], 8)
                sw = _bcast(sin_sb[:, lb, :], 8)
                a_t = stg.tile([128, 8, HD], BF16, tag="ra", bufs=2)
                nc.vector.tensor_mul(a_t, src3, cw)
                b_t = stg.tile([128, 8, HD], BF16, tag="rb", bufs=2)
                nc.vector.tensor_mul(b_t[:, :, 0:32], src3[:, :, 32:64],
                                     _sub(sw, 0, 32))
                nc.vector.tensor_mul(b_t[:, :, 32:64], src3[:, :, 0:32],
                                     _sub(sw, 32, 32))
                nc.vector.tensor_add(dst3, a_t, b_t)

            # per-n-slice shared stat tiles (8 lb units each): the ACT ln/exp
            # rsqrt ops are BATCHED per n-slice (8 lns, then 8 exps) so the
            # sqrt-free scalar stream switches act tables ~2x per slice
            # instead of 2x per unit.
            def emit_unit(n, lb, wqn, ps, copy_eng, nstate):
                """Matmuls + psum evacuation + DVE stats for one (n, lb) unit.

                copy_eng: psum->sbuf staging engine ("act" while ACT is idle
                in wave A, "dve" during wave B when ACT runs exp)."""
                def psum_copy(dst, src):
                    if copy_eng == "act":
                        nc.scalar.copy(dst, src)
                    else:
                        nc.vector.tensor_copy(dst, src)

                for cb in range(NCB):
                    nc.tensor.matmul(
                        ps, lhsT=xr[:, cb, 128 * lb:128 * (lb + 1)],
                        rhs=wqn[:, cb, :],
                        start=(cb == 0), stop=(cb == NCB - 1))
                if n >= 4:
                    c = n - 4
                    psum_copy(vb[:, lb, 8 * c:8 * (c + 1), 0:HD],
                              ps.rearrange("p (h d) -> p h d", d=HD))
                    return
                if lb == 0:
                    nstate['st0'] = stg.tile([128, NLB, 512], BF16,
                                             tag="st08", bufs=2,
                                             name=f"st08_{n}")
                    nstate['ss'] = stg.tile([128, NLB, 8], F32,
                                            tag="ss8", bufs=2,
                                            name=f"ss8_{n}")
                st0 = nstate['st0'][:, lb, :]
                psum_copy(st0, ps)
                s03 = st0.rearrange("p (h d) -> p h d", d=HD)
                sqt = stg.tile([128, 8, HD], BF16, tag="sq", bufs=2)
                nc.vector.tensor_mul(sqt, s03, s03)
                nc.vector.tensor_reduce(nstate['ss'][:, lb, :], sqt,
                                        axis=mybir.AxisListType.X, op=ALU.add)
                if n >= 2:   # k: rope needs no stats, emit immediately
                    c = n - 2
                    rope(s03, kr[:, lb, 512 * c:512 * (c + 1)]
                         .rearrange("p (h d) -> p h d", d=HD),
                         ck_sb, sk_sb, lb)

            def emit_post(n, nstate):
                """Batched rsqrt via exp(-0.5*ln(.)): 8 lns then 8 exps; for
                q-slices also the fq fold + rope (which wait on fq)."""
                if n >= 4:
                    return
                ss8, st08 = nstate['ss'], nstate['st0']
                # rsqrt(ss/64+eps) without Ln: bit-hack log2 on DVE feeds a
                # single Exp (the attention exp table -> no table reloads),
                # then one Newton step recovers ~1e-3 accuracy.
                # fk additionally folds the 1/8 softmax scale into the
                # Newton constants: rsqrt(ss+64eps) = rsqrt(xn)/8.
                xn = stg.tile([128, NLB, 8], F32, tag="xn8", bufs=1)
                nc.vector.tensor_scalar(xn, ss8, 1.0 / HD, EPS,
                                        op0=ALU.mult, op1=ALU.add)
                ib = stg.tile([128, NLB, 8], mybir.dt.int32, tag="ib8",
                              bufs=1)
                nc.vector.tensor_scalar(ib, xn.bitcast(mybir.dt.int32),
                                        0x3F800000, None, op0=ALU.subtract)
                fb = stg.tile([128, NLB, 8], F32, tag="fb8", bufs=1)
                nc.vector.tensor_copy(fb, ib)
                y0 = stg.tile([128, NLB, 8], F32, tag="y08", bufs=1)
                nc.scalar.activation(y0, fb, AF.Exp,
                                     scale=-0.34657359 / (1 << 23),
                                     bias=zero_b)
                u = stg.tile([128, NLB, 8], F32, tag="u8", bufs=1)
                nc.vector.tensor_mul(u, y0, y0)
                u2 = stg.tile([128, NLB, 8], F32, tag="u8b", bufs=1)
                nc.vector.tensor_mul(u2, u, xn)
                w8 = stg.tile([128, NLB, 8], F32, tag="w8", bufs=1)
                if n < 2:
                    nc.vector.tensor_scalar(w8, u2, -0.5, 1.5,
                                            op0=ALU.mult, op1=ALU.add)
                    fq8 = stg.tile([128, NLB, 8], F32, tag="fq8", bufs=2)
                    nc.vector.tensor_mul(fq8, y0, w8)
                    for lb in range(NLB):
                        s03 = st08[:, lb, :].rearrange("p (h d) -> p h d",
                                                       d=HD)
                        st = stg.tile([128, 8, HD], BF16, tag="st", bufs=2)
                        nc.vector.tensor_mul(
                            st, s03, _inner_bcast(fq8[:, lb, :], HD))
                        rope(st, qr[:, lb, 512 * n:512 * (n + 1)]
                             .rearrange("p (h d) -> p h d", d=HD),
                             cq_sb, sq_sb, lb)
                else:
                    c = n - 2
                    nc.vector.tensor_scalar(w8, u2, -0.0625, 0.1875,
                                            op0=ALU.mult, op1=ALU.add)
                    nc.vector.tensor_mul(fk_all[:, :, 8 * c:8 * (c + 1)],
                                         y0, w8)

            def emit_transp_psT(src, dstT, hc, psT, eng="dve"):
                """Wave-A transposes via small dedicated psum pool. Later
                pairs evacuate on ACT (idle during wave A) so the DVE
                backlog does not hold the psum-bank WAR release that gates
                the first attention matmuls."""
                for half in range(2):
                    pt = psT.tile([128, 4, 128], BF16, tag="pt")
                    for t in range(4):
                        lc = 4 * half + t
                        nc.tensor.transpose(
                            pt[:, t, :],
                            src[:, lc, 128 * hc:128 * (hc + 1)], ident)
                    dst = dstT[:, hc, 512 * half:512 * (half + 1)]
                    if eng == "act":
                        nc.scalar.copy(dst, pt.rearrange("p a b -> p (a b)"))
                    else:
                        nc.vector.tensor_copy(
                            dst, pt.rearrange("p a b -> p (a b)"))

            def emit_transp_ring(src, dstT, hc, ring):
                """Wave-B transposes through a bitcast view of a ring tile."""
                rt = ring.tile([128, L], F32, tag="ring")
                ptv = rt.bitcast(BF16).rearrange("p (a b) -> p a b", b=128)
                for lc in range(NLB):
                    nc.tensor.transpose(
                        ptv[:, lc, :],
                        src[:, lc, 128 * hc:128 * (hc + 1)], ident)
                nc.vector.tensor_copy(
                    dstT[:, hc, :],
                    ptv[:, 0:NLB, :].rearrange("p a b -> p (a b)"))

            def kz_copy(hc, i):
                kz = kz0 if i == 0 else kz1
                p0 = 64 * i
                def f():
                    nc.vector.tensor_copy(kz[p0:p0 + HD, :],
                                          kT[p0:p0 + HD, hc, :])
                return f

            def build_attn_steps(pairs, ring, ps_o, state,
                                 skip_kz=(), pair_prefix=None):
                """Globally software-pipelined attention: the S/exp stream
                runs 2 j-blocks ahead of the PV stream ACROSS head
                boundaries, so S(h+1,0)/S(h+1,1) issue before PV(h,6)/
                PV(h,7) and the exp cadence never breaks between heads.
                kz staging is prefetched one head ahead (except heads in
                skip_kz, handled externally); each head's normalization is
                displaced into the next head; pair_prefix[hc] closures are
                emitted just before pair hc's first S matmul."""
                heads = [(hc, i) for hc in pairs for i in (0, 1)]
                pair_prefix = pair_prefix or {}
                nh = len(heads)

                def s_exp(hidx, jb):
                    hc, i = heads[hidx]
                    h = 2 * hc + i
                    kz = kz0 if i == 0 else kz1
                    if jb == 0:
                        state[h] = ps_o.tile([HD + 1, L], F32, tag="pso",
                                             name=f"pso_{h}")
                    ps_st = ring.tile([128, L], F32, tag="ring")
                    for hf in range(2):
                        nc.tensor.matmul(
                            ps_st[:, 512 * hf:512 * (hf + 1)],
                            lhsT=kz[:, 128 * jb:128 * (jb + 1)],
                            rhs=qT[:, hc, 512 * hf:512 * (hf + 1)],
                            start=True, stop=True)
                    pt = stg.tile([128, L], BF16, tag="pts", bufs=3,
                                  name=f"pt_{h}_{jb}")
                    nc.scalar.activation(pt, ps_st, AF.Exp,
                                         scale=fk_all[:, jb, h:h + 1])
                    state[(h, jb)] = pt

                def pv(hidx, jb):
                    hc, i = heads[hidx]
                    h = 2 * hc + i
                    pso = state[h]
                    pt = state.pop((h, jb))
                    for hf in range(2):
                        nc.tensor.matmul(
                            pso[:, 512 * hf:512 * (hf + 1)],
                            lhsT=vb[:, jb, h, :],
                            rhs=pt[:, 512 * hf:512 * (hf + 1)],
                            start=(jb == 0), stop=(jb == NJB - 1))

                def fin(hidx):
                    hc, i = heads[hidx]
                    h = 2 * hc + i
                    p0 = 64 * i
                    pso = state[h]
                    srow = stg.tile([1, L], F32, tag="srow", bufs=1)
                    nc.vector.tensor_copy(srow, pso[HD:HD + 1, :])
                    rs = stg.tile([1, L], F32, tag="rs", bufs=1)
                    nc.vector.reciprocal_approx_fast(rs, srow)
                    fsb = stg.tile([HD, L], F32, tag="fsb", bufs=1)
                    nc.gpsimd.partition_broadcast(fsb, rs)
                    nc.vector.tensor_mul(outT[p0:p0 + HD, hc, :],
                                         pso[0:HD, :], fsb)

                def prefix_for(hidx):
                    if hidx >= nh:
                        return []
                    hc, i = heads[hidx]
                    return pair_prefix.get(hc, []) if i == 0 else []

                out = []
                out.extend(prefix_for(0))
                if 0 not in skip_kz:
                    out.append(kz_copy(*heads[0]))
                out.append(lambda: (s_exp(0, 0), s_exp(0, 1)))

                for hidx in range(nh):
                    for jb in range(NJB):
                        extras = []
                        if jb == 0 and hidx >= 1:
                            # normalization of the previous head right after
                            # its last PV: its srow/recip/broadcast chain must
                            # release the pso bank before s_exp(hidx+1, 0)
                            extras.append(lambda hidx=hidx: fin(hidx - 1))
                        if jb == 0 and hidx + 1 < nh and \
                                (hidx + 1) not in skip_kz:
                            extras.append(kz_copy(*heads[hidx + 1]))
                        tgt = hidx * NJB + jb + 2
                        t_h, t_jb = tgt // NJB, tgt % NJB
                        if t_jb == 0 and t_h < nh:
                            extras.extend(prefix_for(t_h))

                        def step(hidx=hidx, jb=jb, extras=tuple(extras),
                                 t_h=t_h, t_jb=t_jb):
                            for e in extras:
                                e()
                            pv(hidx, jb)
                            if t_h < nh:
                                s_exp(t_h, t_jb)
                        out.append(step)
                out.append(lambda: fin(nh - 1))
                return out

            def wqn_dma(n):
                wqn = stg.tile([128, NCB, 512], BF16, tag="wqn", bufs=2)
                nc.sync.dma_start(out=wqn, in_=wq[:, n, :, :])
                return wqn

            def interleave(asteps, bwork):
                emitted = 0
                for idx, s in enumerate(asteps):
                    s()
                    target = (idx + 1) * len(bwork) // len(asteps)
                    while emitted < target:
                        bwork[emitted]()
                        emitted += 1
                while emitted < len(bwork):
                    bwork[emitted]()
                    emitted += 1

            # ---------------- wave A: heads 0-7 qkv + transposes ----------------
            with nc.named_scope("waveA"), \
                 tc.tile_pool(name="psQA", bufs=4, space="PSUM") as psQA, \
                 tc.tile_pool(name="trA", bufs=2, space="PSUM") as trA:
                def unitA(n, lb, wqn, ns):
                    ps = psQA.tile([128, 512], F32, tag="ps")
                    emit_unit(n, lb, wqn, ps, "act", ns)
                for n in (0, 2):
                    wqn = wqn_dma(n)
                    if n == 0:
                        table_dmas()
                    ns = {}
                    for lb in range(NLB):
                        unitA(n, lb, wqn, ns)
                    emit_post(n, ns)
                wqn = wqn_dma(4)
                for lb in range(NLB):
                    unitA(4, lb, wqn, {})
                for hc in range(4):
                    eng = "dve" if hc < 2 else "act"
                    emit_transp_psT(qr, qT, hc, trA, eng)
                    emit_transp_psT(kr, kT, hc, trA, eng)
                    if hc == 0:
                        # stage pair 0's kz before the remaining transpose
                        # evacuations queue up on DVE
                        kz_copy(0, 0)()
                        kz_copy(0, 1)()

            # ---------------- pipelined attention ----------------
            state = {}
            with tc.tile_pool(name="ring", bufs=2, space="PSUM") as ring, \
                 tc.tile_pool(name="ps_o", bufs=2, space="PSUM") as ps_o:

                # attn pairs 0-3 || wave B qkv + transposes
                with nc.named_scope("attnA"):
                    asteps = build_attn_steps(range(4), ring, ps_o, state,
                                              skip_kz=(0, 1))

                    wq_box = {}
                    rt_box = {}
                    def unitB(n, lbp, half, ns):
                        # half-granular filler: ~8 matmuls per closure keeps
                        # the injected PE bursts shorter than one exp
                        if half == 0:
                            rt_box['t'] = ring.tile([128, L], F32,
                                                    tag="ring", name="rtB")
                        emit_unit(n, 2 * lbp + half, wq_box['v'],
                                  rt_box['t'][:, 512 * half:512 * (half + 1)],
                                  "dve", ns)

                    bwork = []
                    for n in (1, 3, 5):
                        ns = {}
                        bwork.append(lambda n=n: wq_box.__setitem__(
                            'v', wqn_dma(n)))
                        for lbp in range(NLB // 2):
                            for half in range(2):
                                bwork.append(
                                    lambda n=n, lbp=lbp, half=half, ns=ns:
                                    unitB(n, lbp, half, ns))
                        bwork.append(lambda n=n, ns=ns: emit_post(n, ns))
                        if n == 1:
                            bwork.append(lambda: emit_transp_ring(
                                qr, qT, 4, ring))
                        elif n == 3:
                            bwork.append(lambda: emit_transp_ring(
                                kr, kT, 4, ring))
                    interleave(asteps, bwork)

                # attn pairs 4-7 || proj partials (cb 0-3)
                with nc.named_scope("attnB"):
                    # pairs 5-7 transposes run here as extra PE filler, each
                    # just before the pair that consumes it (followed by its
                    # kz staging so DVE ordering stays consistent)
                    prefix = {}
                    for hc in range(5, 8):
                        prefix[hc] = [
                            lambda hc=hc: emit_transp_ring(qr, qT, hc, ring),
                            lambda hc=hc: emit_transp_ring(kr, kT, hc, ring),
                            kz_copy(hc, 0)]
                    # head indices of (5,0),(6,0),(7,0) within pairs 4-7
                    asteps = build_attn_steps(range(4, 8), ring, ps_o, state,
                                              skip_kz=(2, 4, 6),
                                              pair_prefix=prefix)

                    wp_box = {}
                    def wpn_dma(hf):
                        # reuses the wqn ring (same shape/tag) - wq is done
                        wpn = stg.tile([128, NCB, 512], BF16, tag="wqn",
                                       bufs=2)
                        nc.sync.dma_start(out=wpn, in_=wp[:, hf, :, :])
                        wp_box[hf] = wpn

                    prt_box = {}
                    def projA(hf, lbp, half):
                        if half == 0:
                            prt_box['t'] = ring.tile([128, L], F32,
                                                     tag="ring", name="rtP")
                        rt = prt_box['t']
                        lb = 2 * lbp + half
                        psy = rt[:, 512 * half:512 * (half + 1)]
                        for cb in range(4):
                            nc.tensor.matmul(
                                psy,
                                lhsT=outT[:, cb, 128 * lb:128 * (lb + 1)],
                                rhs=wp_box[hf][:, cb, :],
                                start=(cb == 0), stop=(cb == 3))
                        if half == 1:
                            nc.vector.tensor_copy(
                                yacc[:, hf, 2 * lbp:2 * lbp + 2, :]
                                .rearrange("p a b -> p (a b)"), rt)

                    pwork = [lambda: wpn_dma(0), lambda: wpn_dma(1)]
                    for hf in range(2):
                        for lbp in range(NLB // 2):
                            for half in range(2):
                                pwork.append(lambda hf=hf, lbp=lbp, half=half:
                                             projA(hf, lbp, half))
                    interleave(asteps, pwork)

                # proj cb 4-7 + combine with partials
                with nc.named_scope("proj"):
                    for hf in range(2):
                        for lbp in range(NLB // 2):
                            rt = ring.tile([128, L], F32, tag="ring")
                            for half in range(2):
                                lb = 2 * lbp + half
                                psy = rt[:, 512 * half:512 * (half + 1)]
                                for cb in range(4, NCB):
                                    nc.tensor.matmul(
                                        psy,
                                        lhsT=outT[:, cb, 128 * lb:128 * (lb + 1)],
                                        rhs=wp_box[hf][:, cb, :],
                                        start=(cb == 4), stop=(cb == NCB - 1))
                            # drain in 512-col halves: add+DMA of half 0
                            # overlaps the adds of half 1
                            ysb = stg.tile([128, L], F32, tag="ysb", bufs=2)
                            for half in range(2):
                                sl = slice(512 * half, 512 * (half + 1))
                                nc.vector.tensor_add(
                                    ysb[:, sl], rt[:, sl],
                                    yacc[:, hf, 2 * lbp + half, :])
                                nc.sync.dma_start(
                                    out=y[128 * (2 * lbp + half):
                                          128 * (2 * lbp + half + 1),
                                          512 * hf:512 * (hf + 1)],
                                    in_=ysb[:, sl])

                if DEBUG_DUMP:
                    with nc.named_scope("dbg"):
                        for nm, sb in [("d_qr", qr), ("d_kr", kr),
                                       ("d_qT", qT), ("d_kT", kT),
                                       ("d_vb", vb), ("d_fk", fk_all),
                                       ("d_outT", outT), ("d_yacc", yacc),
                                       ("d_kz0", kz0), ("d_kz1", kz1)]:
                            nc.sync.dma_start(out=dbg[nm][:], in_=sb)

    nc.compile()
    return nc


def _get_nc():
    global _nc_cache
    if _nc_cache is None:
        _nc_cache = build_nc()
    return _nc_cache


def _fold(cos, sin, w):
    cosW = cos * w[None, :]
    w_rot = np.concatenate([w[32:], w[:32]])
    sinW = (sin * w_rot[None, :]).copy()
    sinW[:, :32] *= -1.0
    return cosW, sinW


def _host_prep(x, cos, sin, w_qkv, w_proj, q_norm_w, k_norm_w):
    bf = ml_dtypes.bfloat16
    x = np.asarray(x, dtype=np.float32)
    cos = np.asarray(cos, dtype=np.float32)
    sin = np.asarray(sin, dtype=np.float32)
    w_qkv = np.asarray(w_qkv, dtype=np.float32)
    w_proj = np.asarray(w_proj, dtype=np.float32)
    q_norm_w = np.asarray(q_norm_w, dtype=np.float32)
    k_norm_w = np.asarray(k_norm_w, dtype=np.float32)

    def swz_w(w, nslices):
        # (C, n*512) -> (128, n, NCB, 512): [p, n, cb, j] = w[128cb+p, 512n+j]
        return np.ascontiguousarray(
            w.reshape(NCB, 128, nslices, 512).transpose(1, 2, 0, 3)).astype(bf)

    def swz_tab(t):
        # (L, HD) -> (128, NLB, HD): [p, lc, j] = t[128lc+p, j]
        return np.ascontiguousarray(
            t.reshape(NLB, 128, HD).transpose(1, 0, 2)).astype(bf)

    wqT = swz_w(np.ascontiguousarray(w_qkv.T), 6)
    wpT = swz_w(np.ascontiguousarray(w_proj.T), 2)
    cqt, sqt = _fold(cos, sin, q_norm_w)
    ckt, skt = _fold(cos, sin, k_norm_w)
    cqt, sqt, ckt, skt = map(swz_tab, (cqt, sqt, ckt, skt))

    in_maps = []
    for b in range(N_CORES):
        # (C, L) -> (128, NCB, L): [p, cb, l] = x[b].T[128cb+p, l]
        xs = np.ascontiguousarray(
            x[b].T.reshape(NCB, 128, L).transpose(1, 0, 2)).astype(bf)
        in_maps.append({
            "xT": xs, "wq": wqT, "wp": wpT,
            "cq": cqt, "sq": sqt, "ck": ckt, "sk": skt,
        })
    return in_maps


def kernel(x, cos, sin, w_qkv, w_proj, q_norm_w, k_norm_w, _trace=False):
    global _last_results
    nc = _get_nc()
    in_maps = _host_prep(x, cos, sin, w_qkv, w_proj, q_norm_w, k_norm_w)
    r = run_bass_kernel_spmd(nc, in_maps, list(range(N_CORES)), trace=_trace)
    _last_results = r
    return np.stack([r.results[b]["y"] for b in range(N_CORES)], axis=0)
